# revision 1
# baseline (speedup 1.0000x reference)
import sys

sys.path.insert(0, "/opt/trn_rl_repo")

import os
import numpy as np
import ml_dtypes

import concourse.bass as bass
import concourse.mybir as mybir
import concourse.tile as tile
from concourse import bacc
from concourse.bass_utils import run_bass_kernel_spmd
from concourse.masks import make_identity

B, S, D, H = 4, 4096, 1024, 64
QW = 512                      # q-chunk width
NQ = 4                        # q-chunk slots per core
POS = [(0, 3, 4, 7), (1, 2, 5, 6)]   # q-chunk positions per core class
T = (8, 16, 24, 32)           # k-tiles (128 wide) per slot in the uniform graph
NMASK = 8                     # last NMASK tiles of each slot get the causal mask
NKT = S // 128                # 32 k tiles
ND = D // 128                 # 8 d-tiles

BF = mybir.dt.bfloat16
F32 = mybir.dt.float32

_cache = {}


def _build():
    nc = bacc.Bacc("TRN2", target_bir_lowering=False, debug=False, num_devices=8)

    # host pre-swizzled inputs: partition-major layouts so each load is ONE DMA
    xt3 = nc.dram_tensor("xt3", [128, ND, S], BF, kind="ExternalInput").ap()
    # slot-1/2/3 q-chunk blocks, loaded early so no qproj waits on the
    # main chunk stream
    qx = nc.dram_tensor("qx", [128, ND, 3 * QW], BF, kind="ExternalInput").ap()
    wall = nc.dram_tensor("wall", [128, ND * 256], BF, kind="ExternalInput").ap()
    misc = nc.dram_tensor("misc", [128, NQ * NMASK + QW], F32, kind="ExternalInput").ap()
    o = nc.dram_tensor("o", [NQ, H + 1, QW], BF, kind="ExternalOutput").ap()

    with tile.TileContext(nc) as tc:
        with (
            tc.tile_pool(name="persist", bufs=1) as pp,
            tc.tile_pool(name="xin", bufs=1) as xp,
            tc.tile_pool(name="estage", bufs=4) as ep,
            tc.tile_pool(name="vstage", bufs=3) as vsp,
            tc.tile_pool(name="ostage", bufs=2) as osp,
            tc.tile_pool(name="zpsum", bufs=2, space="PSUM") as zp,
            tc.tile_pool(name="opsum", bufs=1, space="PSUM") as op_,
            tc.tile_pool(name="projpsum", bufs=2, space="PSUM") as prp,
            tc.tile_pool(name="vtpsum", bufs=1, space="PSUM") as vtp,
        ):
            # ---- persistent tiles ----
            wall_sb = pp.tile([128, ND * 256], BF, tag="wall")
            misc_sb = pp.tile([128, NQ * NMASK + QW], F32, tag="misc")
            tcol_sb = misc_sb[:, 0:NQ * NMASK]
            iq_sb = misc_sb[:, NQ * NMASK:]
            ident = pp.tile([64, 64], BF, tag="ident")
            qT2 = pp.tile([128, NQ * QW], BF, tag="qT2")
            kT2 = pp.tile([128, S], BF, tag="kT2")
            vws = pp.tile([128, NKT * (H + 1)], BF, tag="vws")
            # all 32 causal masks, grouped per (slot, pair): [128, 1024] each
            msk_all = pp.tile([128, NQ * (NMASK // 2) * 1024], BF, tag="mskall")
            # mini e-queue: slot-3 pairs 0-3 exp'd early inside attn2's holes
            eq3 = pp.tile([128, 4 * 1024], BF, tag="eq3")
            xtall = xp.tile([128, ND, S], BF, tag="xtall")
            qxall = xp.tile([128, ND, 3 * QW], BF, tag="qxall")

            # ---- input DMAs: one trigger per logical block, consumption order
            nc.sync.dma_start(misc_sb[:], misc[:])
            nc.sync.dma_start(wall_sb[:], wall[:])
            # first two chunks split by d-halves so the kv accumulation can
            # start before the full chunk lands
            nc.sync.dma_start(xtall[:, 0:4, 0:512], xt3[:, 0:4, 0:512])
            nc.sync.dma_start(xtall[:, 4:8, 0:512], xt3[:, 4:8, 0:512])
            nc.sync.dma_start(qxall[:, :, 0:QW], qx[:, :, 0:QW])
            nc.sync.dma_start(xtall[:, 0:4, 512:1024], xt3[:, 0:4, 512:1024])
            nc.sync.dma_start(xtall[:, 4:8, 512:1024], xt3[:, 4:8, 512:1024])
            nc.sync.dma_start(qxall[:, :, QW:3 * QW], qx[:, :, QW:3 * QW])
            for c in range(2, 2 * NQ):
                nc.sync.dma_start(xtall[:, :, c * 512:(c + 1) * 512],
                                  xt3[:, :, c * 512:(c + 1) * 512])

            make_identity(nc, ident[:])
            # hold the PE busy (HAM warm) while the first x chunks stream in
            wtile = vtp.tile([128, 64], F32, tag="vt", name="warmps")
            for i in range(72):
                nc.tensor.matmul(wtile[0:64, :], wall_sb[:, 0:64],
                                 wall_sb[:, 64:128], start=True, stop=True)
            nc.gpsimd.memset(vws[:], 1.0)
            # warm the ACT exp table early
            warm = ep.tile([128, 1], BF, tag="warm")
            nc.scalar.activation(warm[:], misc_sb[:, 0:1],
                                 mybir.ActivationFunctionType.Exp)

            def emit_masks(s_):
                # mask generation on the (otherwise idle) gpsimd engine
                for mp in range(NMASK // 2):
                    for h2 in range(2):
                        m = NMASK * s_ + 2 * mp + h2
                        base = (s_ * (NMASK // 2) + mp) * 1024 + h2 * 512
                        nc.vector.tensor_scalar(
                            msk_all[:, base:base + 512], iq_sb,
                            tcol_sb[:, m:m + 1], None,
                            mybir.AluOpType.is_ge)

            POSU = (0, 3, 4, 7)   # uniform q-chunk positions in permuted order

            _qps = {}

            def qproj_a(w):
                ps = prp.tile([128, 512], F32, tag="proj", name=f"qps{w}")
                _qps[w] = ps
                for d in range(4):
                    if w >= 1:
                        rhs = qxall[:, d, (w - 1) * QW:w * QW]
                    else:
                        rhs = xtall[:, d, 0:QW]
                    nc.tensor.matmul(ps[:], wall_sb[:, d * 256:d * 256 + 128],
                                     rhs, start=(d == 0), stop=False)

            def qproj_b(w):
                ps = _qps.pop(w)
                for d in range(4, ND):
                    if w >= 1:
                        rhs = qxall[:, d, (w - 1) * QW:w * QW]
                    else:
                        rhs = xtall[:, d, 0:QW]
                    nc.tensor.matmul(ps[:], wall_sb[:, d * 256:d * 256 + 128],
                                     rhs, start=False, stop=(d == ND - 1))
                # rows 0-63 and 64-127 both hold q^T (duplicated weights)
                nc.vector.tensor_copy(qT2[:, w * QW:(w + 1) * QW], ps[:])

            def qproj(w):
                qproj_a(w)
                qproj_b(w)

            _kvps = {}
            _kvvst = {}

            def kv_a(sc):
                ps = prp.tile([128, 512], F32, tag="proj", name=f"kvps{sc}")
                _kvps[sc] = ps
                for d in range(4):
                    nc.tensor.matmul(ps[:], wall_sb[:, d * 256 + 128:d * 256 + 256],
                                     xtall[:, d, sc * 512:(sc + 1) * 512],
                                     start=(d == 0), stop=False)

            def kv_b(sc):
                ps = _kvps.pop(sc)
                for d in range(4, ND):
                    nc.tensor.matmul(ps[:], wall_sb[:, d * 256 + 128:d * 256 + 256],
                                     xtall[:, d, sc * 512:(sc + 1) * 512],
                                     start=False, stop=(d == ND - 1))
                nc.vector.tensor_copy(kT2[0:64, sc * 512:(sc + 1) * 512], ps[0:64, :])
                # duplicate k^T into the upper partition half (for row-tiled QK)
                # via an identity matmul into PE col-group (0,64) — no DMA.
                # kdup borrows the z-pool rotation to keep prp free for proj ps
                kdup = zp.tile([128, 512], F32, tag="z", name=f"kdup{sc}")
                nc.tensor.matmul(kdup[64:128, :], ident[:],
                                 kT2[0:64, sc * 512:(sc + 1) * 512],
                                 start=True, stop=True)
                nc.vector.tensor_copy(kT2[64:128, sc * 512:(sc + 1) * 512],
                                      kdup[64:128, :])
                vstage = vsp.tile([64, 512], BF, tag="vstage", name=f"vst{sc}")
                nc.vector.tensor_copy(vstage[:], ps[64:128, :])
                _kvvst[sc] = vstage

            def kv_c(sc):
                vstage = _kvvst.pop(sc)
                # all 4 transposes into one PSUM tile (68-col stride keeps the
                # matmul writes 8B-aligned), then ONE strided copy into vws
                vt4 = vtp.tile([128, 4 * 68], BF, tag="vt", name=f"vt4_{sc}")
                for t in range(4):
                    nc.tensor.transpose(vt4[:, t * 68:t * 68 + 64],
                                        vstage[:, t * 128:(t + 1) * 128],
                                        ident[:])
                kt0 = 4 * sc
                dst = vws[:, kt0 * (H + 1):(kt0 + 4) * (H + 1)]
                nc.vector.tensor_copy(
                    dst.rearrange("p (t h) -> p t h", h=H + 1)[:, :, 0:H],
                    vt4[:].rearrange("p (t h) -> p t h", h=68)[:, :, 0:H])

            def kv_chunk(sc):
                kv_a(sc)
                kv_b(sc)
                kv_c(sc)

            def z_exp(s_, p, e_ap, name):
                """z matmul pair + exp for slot s_, tile pair p -> e_ap."""
                j0, j1 = 2 * p, 2 * p + 1
                z = zp.tile([128, 1024], F32, tag="z", name=f"z{name}")
                # two K=64 matmuls in different PE row groups -> concurrent
                nc.tensor.matmul(z[:, 0:512],
                                 kT2[0:64, j0 * 128:(j0 + 1) * 128],
                                 qT2[0:64, s_ * QW:(s_ + 1) * QW],
                                 start=True, stop=True)
                nc.tensor.matmul(z[:, 512:1024],
                                 kT2[64:128, j1 * 128:(j1 + 1) * 128],
                                 qT2[64:128, s_ * QW:(s_ + 1) * QW],
                                 start=True, stop=True)
                nc.scalar.activation(e_ap, z[:],
                                     mybir.ActivationFunctionType.Exp,
                                     scale=0.125)

            def pre3(p):
                z_exp(3, p, eq3[:, p * 1024:(p + 1) * 1024], f"pre3_{p}")

            def attn_slot(s_, inject=(), order=None, npre=0):
                ts_ = T[s_]
                np_ = ts_ // 2   # tile pairs
                inj = dict(inject)
                ops = op_.tile([H + 1, 512], F32, tag="oacc", name=f"oacc{s_}")
                if order is None:
                    order = list(range(np_))
                for i, p in enumerate(order):
                    for th in inj.pop(i, ()):
                        th()
                    j0, j1 = 2 * p, 2 * p + 1
                    if p < npre:
                        e_ap = eq3[:, p * 1024:(p + 1) * 1024]
                    else:
                        e = ep.tile([128, 1024], BF, tag="e", name=f"e{s_}_{p}")
                        e_ap = e[:]
                        z_exp(s_, p, e_ap, f"{s_}_{p}")
                        if p >= np_ - NMASK // 2:
                            mp = p - (np_ - NMASK // 2)
                            base = (s_ * (NMASK // 2) + mp) * 1024
                            nc.vector.tensor_tensor(e_ap, e_ap,
                                                    msk_all[:, base:base + 1024],
                                                    mybir.AluOpType.mult)
                    nc.tensor.matmul(ops[:],
                                     vws[:, j0 * (H + 1):(j0 + 1) * (H + 1)],
                                     e_ap[:, 0:512], start=(i == 0), stop=False)
                    nc.tensor.matmul(ops[:],
                                     vws[:, j1 * (H + 1):(j1 + 1) * (H + 1)],
                                     e_ap[:, 512:1024], start=False,
                                     stop=(i == np_ - 1))
                osb = osp.tile([H + 1, 512], BF, tag="osb", name=f"osb{s_}")
                # scalar engine idles at slot boundaries; vector stays free
                # for the next slot's copies
                nc.scalar.copy(osb[:], ops[:])
                nc.sync.dma_start(o[s_], osb[:])

            # wave 0
            emit_masks(0)
            emit_masks(1)
            kv_chunk(0)
            qproj(0)
            # chunk-0 pairs of attn0 start immediately; every later
            # kv/qproj half-chain is injected one-stage-per-pair so no chain
            # exceeds the buffered-exp coverage
            attn_slot(0, inject={
                1: (lambda: kv_a(1), lambda: kv_b(1)),
                2: (lambda: kv_c(1), lambda: qproj_a(1)),
                3: (lambda: qproj_b(1),),
            })
            attn_slot(1, inject={
                1: (lambda: kv_a(2),),
                2: (lambda: kv_b(2),),
                3: (lambda: kv_c(2),),
                4: (lambda: kv_a(3),),
                5: (lambda: kv_b(3),),
                6: (lambda: kv_c(3),),
                7: (lambda: qproj_a(2),),
            })
            attn_slot(2, inject={
                0: (lambda: qproj_b(2),),
                1: (lambda: qproj_a(3),),
                2: (lambda: qproj_b(3),),
                3: (lambda: emit_masks(2), lambda: kv_a(4)),
                4: (lambda: kv_b(4),),
                5: (lambda: kv_c(4),),
                6: (lambda: emit_masks(3), lambda: kv_a(5)),
                7: (lambda: kv_b(5),),
                8: (lambda: kv_c(5), lambda: pre3(0)),
                9: (lambda: pre3(1),),
                10: (lambda: pre3(2),),
                11: (lambda: pre3(3),),
            })
            attn_slot(3, npre=4,
                      order=list(range(8, 16)) + list(range(4, 8))
                      + list(range(4)),
                      inject={
                0: (lambda: kv_a(6),),
                1: (lambda: kv_b(6),),
                2: (lambda: kv_c(6),),
                4: (lambda: kv_a(7),),
                5: (lambda: kv_b(7),),
                6: (lambda: kv_c(7),),
            })

    nc.compile()
    return nc


def _get_nc():
    if "nc" not in _cache:
        _cache["nc"] = _build()
    return _cache["nc"]


def kernel(x, Wk, Wq, Wv):
    x = np.asarray(x, dtype=np.float32)
    Wk = np.asarray(Wk, dtype=np.float32)
    Wq = np.asarray(Wq, dtype=np.float32)
    Wv = np.asarray(Wv, dtype=np.float32)

    nc = _get_nc()

    # wall[p, d*256 + 0:128]   = Wq[d*128+p, :] | Wq  (duplicated)
    # wall[p, d*256 + 128:256] = Wk[d*128+p, 0:64] | Wv
    wqq = np.concatenate([Wq, Wq], axis=1)        # [1024, 128]
    wkv = np.concatenate([Wk, Wv], axis=1)        # [1024, 128]
    wboth = np.concatenate([wqq, wkv], axis=1)    # [1024, 256]
    wall_np = np.ascontiguousarray(
        wboth.reshape(ND, 128, 256).transpose(1, 0, 2).reshape(128, ND * 256)
    ).astype(ml_dtypes.bfloat16)

    iq_np = np.broadcast_to(np.arange(QW, dtype=np.float32), (128, QW))

    xt_b = [np.ascontiguousarray(x[b].T).astype(ml_dtypes.bfloat16)
            for b in range(B)]
    # [D, S] -> [128, ND, S] partition-major swizzle
    xt3_b = [np.ascontiguousarray(xb.reshape(ND, 128, S).transpose(1, 0, 2))
             for xb in xt_b]

    # per-class k-chunk permutation: places each class's q-chunks at the
    # uniform positions (0,3,4,7) while preserving causal prefix coverage
    PERM = [(0, 1, 2, 3, 4, 5, 6, 7), (1, 0, 3, 2, 5, 4, 7, 6)]

    in_maps = []
    for c in range(8):
        b, cls = c >> 1, c & 1
        pos = POS[cls]
        perm = PERM[cls]
        xt3_np = np.concatenate(
            [xt3_b[b][:, :, pc * 512:(pc + 1) * 512] for pc in perm], axis=2)
        qx_np = np.concatenate(
            [xt3_b[b][:, :, pos[w] * QW:(pos[w] + 1) * QW] for w in (1, 2, 3)],
            axis=2)
        tcol_np = np.zeros((128, NQ * NMASK), np.float32)
        krange = np.arange(128, dtype=np.float32)
        for s_ in range(NQ):
            for jj in range(NMASK):
                j = T[s_] - NMASK + jj          # permuted tile index
                orig_tile = perm[j // 4] * 4 + j % 4
                tcol_np[:, NMASK * s_ + jj] = (
                    krange + 128.0 * orig_tile - 512.0 * pos[s_])
        misc_np = np.concatenate([tcol_np, iq_np], axis=1).astype(np.float32)
        in_maps.append({
            "xt3": np.ascontiguousarray(xt3_np),
            "qx": np.ascontiguousarray(qx_np),
            "wall": wall_np,
            "misc": misc_np,
        })

    trace = bool(int(os.environ.get("KERNEL_TRACE", "0")))
    res = run_bass_kernel_spmd(nc, in_maps, core_ids=list(range(8)), trace=trace)
    _cache["last_result"] = res

    out = np.zeros((B, S, H), np.float32)
    for c in range(8):
        b, cls = c >> 1, c & 1
        oc = res.results[c]["o"].astype(np.float32)   # [NQ, 65, 512] bf16
        for s_, p in enumerate(POS[cls]):
            num = oc[s_, 0:H, :]          # [64, 512]
            den = oc[s_, H, :]            # [512]
            out[b, p * QW:(p + 1) * QW, :] = (num / den[None, :]).T
    return out

